# revision 133
# baseline (speedup 1.0000x reference)
"""Trainium2 Bass kernel for nn_BiBoAttention (B=2, S=2048, D=2048, H=16).

Sharding: 8 cores = 2 batches x 4 head-groups (4 heads of 128 dims each).

v3 design (vs v2): every GEMM runs as fp8 DoubleRowSwInterleave matmuls
(0.5 cyc/row, 256-contraction per instruction) in DUAL (hi+lo e4m3)
precision, except scores (bf16) and probs (single fp8 - softmax diffusion
damps probs noise ~2.4x, measured):
- Q/K/V projections: 8+8+8 fc-pair-fold DRS terms (w_hi*x_hi + w_lo*x_hi +
  w_hi*x_lo), host-prepped fold interleaves; V stationary is x-side
  (x8vh/x8vl fold tensors) so out lands [token, (h,hd)].
- V is split on-device into (v_hi, v_lo) fp8 kb-pair interleaves (Act copy +
  DVE subtract of the f32 PSUM) feeding PV-DRS.
- exp() writes probs DIRECTLY as fp8 e4m3 with bias -ln(16) (overflow-safe);
  l = sum_k p via ones-matmuls on the fp8 probs.
- PV: 2 DRS matmuls per kb-pair (v_hi*p + v_lo*p); diagonal pairs narrowed
  to live columns [w0e:512] (scores, template, exp, l, PV all skip the
  fully-masked belt).
- Wo: o8 split to (hi, lo) fp8 head-pair interleaves; 6 DRS matmuls per
  128-outcol block (wo_hi*o_hi + wo_lo*o_hi + wo_hi*o_lo); output written
  TRANSPOSED [D, S] (host re-transposes) so the out-DMA stays contiguous;
  4 outcol blocks batched per DMA.
- Scale plumbing: weights x32 (fp8 range), probs /16 (exp bias), o8 = 32*out,
  final osb copy descales by 1/1024.
- Wo chains paced evenly across the next chunk's score pairs (fills Act-exp
  latency windows); chunk-major x8 dram layout (1KB runs, no small-elem DMA
  penalty).
- Causal mask still accumulated into scores PSUM by the PE via template
  matmuls; l-broadcast via PE transpose + outer-product as in v2.
- Variable out-DMA groups (4,4,4,2,1,1 on the last chunk) shorten the final
  copy->dispatch->transfer tail; PSUM-reading splits/copies balanced across
  Act and DVE (Act is exp-saturated during late-chunk steps).
- Startup stream interleaves wq-lo/wk-lo quarters with the x8/wq-hi
  quarters (B-terms stop stalling); chain copies that drain during the
  exp-saturated last chunk go DVE-only; chain matmuls grouped hp0-first so
  the final drain overlaps the last head's o8 split.
- rel-err ~1.3e-2 (limit 2e-2); cost-model timeline 258.9us vs v2's 320us.
"""
import math
import ml_dtypes
import numpy as np
from contextlib import ExitStack

LOG16 = math.log(16.0)

import concourse.bass as bass
import concourse.bass_isa as bass_isa
import concourse.mybir as mybir
import concourse.tile as tile
from concourse import bacc
from concourse.bass_utils import run_bass_kernel_spmd

F32 = mybir.dt.float32
BF16 = mybir.dt.bfloat16
FP8 = mybir.dt.float8e4
DRS = mybir.MatmulPerfMode.DoubleRowSwInterleave
ALU = mybir.AluOpType
ACTF = mybir.ActivationFunctionType

B = 2
D = 2048
H = 16
HD = 128
P = 128
FC = D // P          # 16 feature chunks
NH = 4               # heads per core
DG = NH * HD         # 512 group width
NCORES = 8
ROPE_THETA = 10000.0
NEG = -200.0         # additive mask value (exp -> 0; fits fp8 e4m3)


def build_program(S, causal):
    KQ = S // 512            # q-groups of 512
    NKB = S // P             # 128-token k-blocks
    nc = bacc.Bacc("TRN2", target_bir_lowering=False, debug=False,
                   num_devices=NCORES)

    NKBALL = S // P
    x8_d = nc.declare_dram_parameter("x8c", [S // 512, P, FC, 2, 512], FP8,
                                     isOutput=False)
    x8vh_d = nc.declare_dram_parameter("x8vh", [P, FC // 2, NKBALL, P, 2], FP8,
                                       isOutput=False)
    x8vl_d = nc.declare_dram_parameter("x8vl", [P, FC // 2, NKBALL, P, 2], FP8,
                                       isOutput=False)
    wqfh_d = nc.declare_dram_parameter("wqfh", [P, FC // 2, NH, 2 * HD], FP8,
                                       isOutput=False)
    wqfl_d = nc.declare_dram_parameter("wqfl", [P, FC // 2, NH, 2 * HD], FP8,
                                       isOutput=False)
    wkfh_d = nc.declare_dram_parameter("wkfh", [P, FC // 2, NH, 2 * HD], FP8,
                                       isOutput=False)
    wkfl_d = nc.declare_dram_parameter("wkfl", [P, FC // 2, NH, 2 * HD], FP8,
                                       isOutput=False)
    wv8h_d = nc.declare_dram_parameter("wv8h", [P, FC, DG], FP8, isOutput=False)
    wv8l_d = nc.declare_dram_parameter("wv8l", [P, FC, DG], FP8, isOutput=False)
    wo8h_d = nc.declare_dram_parameter("wo8h", [P, 2, FC, P, 2], FP8,
                                       isOutput=False)
    wo8l_d = nc.declare_dram_parameter("wo8l", [P, 2, FC, P, 2], FP8,
                                       isOutput=False)
    cos_d = nc.declare_dram_parameter("cos", [P, S], BF16, isOutput=False)
    sin_d = nc.declare_dram_parameter("sin", [P, S], BF16, isOutput=False)
    id_d = nc.declare_dram_parameter("identb", [P, P], BF16, isOutput=False)
    ones_d = nc.declare_dram_parameter("onesb8", [P, 1], FP8, isOutput=False)
    idf_d = nc.declare_dram_parameter("identf", [P, P], F32, isOutput=False)
    onesf_d = nc.declare_dram_parameter("onesf4", [1, P], BF16, isOutput=False)
    if causal:
        tm_d = nc.declare_dram_parameter("tmpl", [P, 4, 512], FP8,
                                         isOutput=False)
    out_d = nc.declare_dram_parameter("out", [P, FC, S], BF16, isOutput=True)

    with tile.TileContext(nc) as tc, ExitStack() as octx:
        sb = octx.enter_context(tc.tile_pool(name="sb", bufs=1))
        xp = octx.enter_context(tc.tile_pool(name="xp", bufs=2))
        rp = octx.enter_context(tc.tile_pool(name="rp", bufs=2))
        p8p = octx.enter_context(tc.tile_pool(name="p8p", bufs=5))
        lvp = octx.enter_context(tc.tile_pool(name="lvp", bufs=2))
        o8p = octx.enter_context(tc.tile_pool(name="o8p", bufs=2))
        obp = octx.enter_context(tc.tile_pool(name="obp", bufs=3))
        qtp = octx.enter_context(tc.tile_pool(name="qtp", bufs=2))
        osp = octx.enter_context(tc.tile_pool(name="osp", bufs=4))
        ps = octx.enter_context(tc.tile_pool(name="ps", bufs=1, space="PSUM"))

        # ---- persistent SBUF ----
        # first matmul chain needs x8 chunk 0 + wqf; split those DMAs so the
        # fcp<4 part of the first chain can start early
        x80 = xp.tile([P, FC, 2, 512], FP8, tag="x8", bufs=1)
        wqfh_sb = sb.tile([P, FC // 2, NH, 2 * HD], FP8, tag="wqfh")
        wqfl_sb = sb.tile([P, FC // 2, NH, 2 * HD], FP8, tag="wqfl")
        for q4 in range(0, 4):
            fs = slice(q4 * FC // 4, (q4 + 1) * FC // 4)
            fps = slice(q4 * FC // 8, (q4 + 1) * FC // 8)
            nc.sync.dma_start(x80[:, fs], x8_d[0, :, fs])
            nc.sync.dma_start(wqfh_sb[:, fps], wqfh_d[:, fps])
            nc.sync.dma_start(wqfl_sb[:, fps], wqfl_d[:, fps])
        cos0 = rp.tile([P, 512], BF16, tag="cos")
        nc.sync.dma_start(cos0[:], cos_d[:, 0:512])
        sin0 = rp.tile([P, 512], BF16, tag="sin")
        nc.sync.dma_start(sin0[:], sin_d[:, 0:512])
        biasv = sb.tile([P, 1], F32, tag="biasv")
        nc.gpsimd.memset(biasv[:], -LOG16)
        sc32 = sb.tile([P, 1], F32, tag="sc32")
        nc.gpsimd.memset(sc32[:], 1.0 / 1024.0)
        wkfh_sb = sb.tile([P, FC // 2, NH, 2 * HD], FP8, tag="wkfh")
        wkfl_sb = sb.tile([P, FC // 2, NH, 2 * HD], FP8, tag="wkfl")
        for q4 in range(0, 4):
            fps = slice(q4 * FC // 8, (q4 + 1) * FC // 8)
            nc.sync.dma_start(wkfh_sb[:, fps], wkfh_d[:, fps])
            nc.sync.dma_start(wkfl_sb[:, fps], wkfl_d[:, fps])
        wv8h_sb = sb.tile([P, FC, DG], FP8, tag="wv8h")
        wv8l_sb = sb.tile([P, FC, DG], FP8, tag="wv8l")
        identb = sb.tile([P, P], BF16, tag="identb")
        nc.sync.dma_start(identb[:], id_d[:])
        onesb = sb.tile([P, 1], FP8, tag="onesb")
        nc.sync.dma_start(onesb[:], ones_d[:])
        identf = sb.tile([P, P], F32, tag="identf")
        nc.sync.dma_start(identf[:], idf_d[:])
        onesf = sb.tile([1, P], BF16, tag="onesf")
        nc.sync.dma_start(onesf[:], onesf_d[:])
        if causal:
            tmpl = sb.tile([P, 4, 512], FP8, tag="tmpl")
            nc.sync.dma_start(tmpl[:], tm_d[:])
        kt = sb.tile([P, NH, S], BF16, tag="kt")
        vhi8 = sb.tile([P, NH, NKB // 2, HD, 2], FP8, tag="vhi8")
        vlo8 = sb.tile([P, NH, NKB // 2, HD, 2], FP8, tag="vlo8")
        wo8h_sb = sb.tile([P, 2, FC, P, 2], FP8, tag="wo8h")
        wo8l_sb = sb.tile([P, 2, FC, P, 2], FP8, tag="wo8l")

        wo_started = [False]
        chunk_tiles = {0: (x80, cos0, sin0)}
        xv_tiles = {}
        qt_tiles = {}

        def prefetch_xt(c):
            if c < KQ and c not in chunk_tiles:
                x8_n = xp.tile([P, FC, 2, 512], FP8, tag="x8", bufs=1)
                nc.sync.dma_start(x8_n[:], x8_d[c])
                cos_n = rp.tile([P, 512], BF16, tag="cos")
                nc.sync.dma_start(cos_n[:], cos_d[:, c * 512:(c + 1) * 512])
                sin_n = rp.tile([P, 512], BF16, tag="sin")
                nc.sync.dma_start(sin_n[:], sin_d[:, c * 512:(c + 1) * 512])
                chunk_tiles[c] = (x8_n, cos_n, sin_n)

        def prefetch_xv(c):
            if c < KQ and c not in xv_tiles:
                if c == 0:
                    nc.sync.dma_start(wv8h_sb[:], wv8h_d[:])
                    nc.sync.dma_start(wv8l_sb[:], wv8l_d[:])
                xvh_n = xp.tile([P, FC // 2, 4, P, 2], FP8, tag="xvh", bufs=1)
                nc.sync.dma_start(xvh_n[:], x8vh_d[:, :, c * 4:(c + 1) * 4])
                xvl_n = xp.tile([P, FC // 2, 4, P, 2], FP8, tag="xvl", bufs=1)
                nc.sync.dma_start(xvl_n[:], x8vl_d[:, :, c * 4:(c + 1) * 4])
                xv_tiles[c] = (xvh_n, xvl_n)

        def emit_phase1_chunk(c):
            t0 = c * 512
            prefetch_xt(c)
            prefetch_xv(c)
            x8_sb, csl, ssl = chunk_tiles.pop(c)
            xvh_sb, xvl_sb = xv_tiles.pop(c)
            prefetch_xt(c + 1)
            prefetch_xv(c + 1)
            qt_c = qtp.tile([P, NH, 512], BF16, tag="qt",
                            bufs=(2 if causal else KQ))
            qt_tiles[c] = qt_c
            sq = 1.0 / (32.0 * math.sqrt(HD))
            sk = 1.0 / 32.0
            NFP = FC // 2
            terms = ((True, 0, True, False), (False, 0, False, False),
                     (True, 1, False, True))  # (use_hi, x-slot, start, stop)

            def proj_mm(qk_ap, wh_sb, wl_sb, h, fcp, term):
                use_hi, slot, st_, sp_ = terms[term]
                w_sb_ = wh_sb if use_hi else wl_sb
                nc.tensor.matmul(qk_ap, w_sb_[:, fcp, h, :],
                                 x8_sb[:, 2 * fcp:2 * fcp + 2, slot, :],
                                 start=(st_ and fcp == 0),
                                 stop=(sp_ and fcp == NFP - 1),
                                 perf_mode=DRS)

            def rope(qk_ap, h, side, ss):
                # RoPE: ro = s*qk*cos + rot_half(s*qk)*sin
                roc = rp.tile([P, 512], BF16, tag="roc")
                rot = rp.tile([P, 512], BF16, tag="rot")
                nc.vector.scalar_tensor_tensor(
                    roc[:], qk_ap, ss, csl[:], op0=ALU.mult, op1=ALU.mult)
                nc.vector.scalar_tensor_tensor(
                    rot[0:64, :], qk_ap[64:128, :], -ss,
                    ssl[0:64, :], op0=ALU.mult, op1=ALU.mult)
                nc.vector.scalar_tensor_tensor(
                    rot[64:128, :], qk_ap[0:64, :], ss,
                    ssl[64:128, :], op0=ALU.mult, op1=ALU.mult)
                if side == 0:
                    nc.gpsimd.tensor_add(qt_c[:, h, :], roc[:], rot[:])
                else:
                    nc.gpsimd.tensor_add(kt[:, h, t0:t0 + 512],
                                         roc[:], rot[:])

            for side, (wh_sb, wl_sb, ss) in enumerate(
                    ((wqfh_sb, wqfl_sb, sq), (wkfh_sb, wkfl_sb, sk))):
                for h in range(NH):
                    qk = ps.tile([P, 512], F32, tag="acc", bufs=3)
                    for term in range(3):
                        for fcp in range(NFP):
                            proj_mm(qk[:], wh_sb, wl_sb, h, fcp, term)
                    rope(qk[:], h, side, ss)
            def emit_v(kb2, c=c, xvh_sb=xvh_sb, xvl_sb=xvl_sb):
                kb = c * 4 + kb2
                pv = ps.tile([P, NH, HD], F32, tag="acc", bufs=3)
                for fcp in range(NFP):
                    nc.tensor.matmul(pv[:], xvh_sb[:, fcp, kb2],
                                     wv8h_sb[:, 2 * fcp:2 * fcp + 2, :],
                                     start=(fcp == 0), stop=False,
                                     perf_mode=DRS)
                for fcp in range(NFP):
                    nc.tensor.matmul(pv[:], xvh_sb[:, fcp, kb2],
                                     wv8l_sb[:, 2 * fcp:2 * fcp + 2, :],
                                     start=False, stop=False, perf_mode=DRS)
                for fcp in range(NFP):
                    nc.tensor.matmul(pv[:], xvl_sb[:, fcp, kb2],
                                     wv8h_sb[:, 2 * fcp:2 * fcp + 2, :],
                                     start=False, stop=(fcp == NFP - 1),
                                     perf_mode=DRS)
                hi = vhi8[:, :, kb // 2, :, kb % 2]
                nc.vector.tensor_copy(hi, pv[:])
                nc.vector.tensor_sub(vlo8[:, :, kb // 2, :, kb % 2], pv[:], hi)
            for kb2 in range(4):
                pending_v.append(lambda kb2=kb2: emit_v(kb2))
            if not wo_started[0]:
                wo_started[0] = True
                nc.sync.dma_start(wo8h_sb[:], wo8h_d[:])
                nc.sync.dma_start(wo8l_sb[:], wo8l_d[:])

        # ---- phase 2 ----
        pend = []          # pipelined (state dict) entries, depth 1
        wo_queue = []      # deferred Wo chain closures
        pending_v = []     # deferred V-projection chains of the current chunk
        step_state = {}

        def emit_scores_exp(h, I, pp, npair):
            s2 = ps.tile([P, 2, 512], F32, tag="s2", bufs=2)
            qsl = qt_tiles[I][:, h, :]
            diag = causal and (2 * pp >= 4 * I)
            # w0e: columns [0, w0e) of BOTH slots are fully masked for a diag
            # pair - never computed, never exp'd, and PV/l skip them.
            w0e = (2 * pp - 4 * I) * P if diag else 0
            for t in range(2):
                kb = 2 * pp + t
                if not diag:
                    nc.tensor.matmul(s2[:, t, :], kt[:, h, kb * P:(kb + 1) * P],
                                     qsl, start=True, stop=True)
                else:
                    # diag block: scores on [w0:512] only; the template matmul
                    # supplies -200 on [w0e:w0] (slot 1's leading masked belt)
                    # plus the triangle band [w0:w].
                    kbl = kb - 4 * I
                    w0 = kbl * P
                    w = w0 + P
                    nc.tensor.matmul(s2[:, t, w0:512],
                                     kt[:, h, kb * P:(kb + 1) * P],
                                     qsl[:, w0:512], start=True, stop=False)
                    nc.tensor.matmul(s2[:, t, w0e:w], identb[:],
                                     tmpl[:, kbl, w0e:w], start=False,
                                     stop=True, skip_group_check=True)
            p8 = p8p.tile([P, 2, 512], FP8, tag="p8")
            if w0e > 0:
                nc.scalar.activation(p8[:, :, w0e:512], s2[:, :, w0e:512],
                                     ACTF.Exp, bias=biasv[:])
            else:
                nc.scalar.activation(p8[:], s2[:], ACTF.Exp, bias=biasv[:])
            return p8, w0e

        def emit_lpv(e):
            h, I, pp, p8, npair = e["h"], e["I"], e["pp"], e["p8"], e["np"]
            w0e = e["w0e"]
            st = step_state[(h, I)]
            if pp == 0:
                l4_t = ps.tile([P, 512], F32, tag="l4", bufs=1)
                ot_t = ps.tile([P, 512], F32, tag="acc", bufs=3)
                st["l4"] = l4_t
                st["ot"] = ot_t
            first = (pp == 0)
            last = (pp == npair - 1)
            # l via ~1-cycle PE ones-matmuls. Only the very first matmul of the
            # step carries start=True: the executor's pending-zero marking is
            # bank-granular, so that single start arms the whole l4 bank and
            # each column's first write lands as a fresh value.
            # Quarters entirely inside [0, w0e) hold garbage p8 - skip (their
            # true contribution is zero).
            for t in range(2):
                for cq in range(4):
                    if (cq + 1) * P <= w0e:
                        continue
                    nc.tensor.matmul(
                        st["l4"][:, cq:cq + 1],
                        p8[:, t, cq * P:(cq + 1) * P], onesb[:],
                        start=(first and t == 0 and cq == 0),
                        stop=(last and t == 1 and cq == 3),
                        skip_group_check=True)
            nc.tensor.matmul(st["ot"][:, w0e:512], vhi8[:, h, pp, :, :],
                             p8[:, :, w0e:512],
                             start=first, stop=False, perf_mode=DRS,
                             skip_group_check=True)
            nc.tensor.matmul(st["ot"][:, w0e:512], vlo8[:, h, pp, :, :],
                             p8[:, :, w0e:512],
                             start=False, stop=last, perf_mode=DRS,
                             skip_group_check=True)
            if last:
                emit_norm_tail_a(h, I)

        tailb_queue = []

        def emit_norm_tail_a(h, I):
            # 1/l, then transpose it into the spare columns of the l4 bank at
            # partition quadrants (no DMA round-trip), copy to SBUF once
            st = step_state[(h, I)]
            linv4 = lvp.tile([P, 4], F32, tag="linv4")
            nc.vector.reciprocal(linv4[:], st["l4"][:, 0:4])
            for cq in range(4):
                nc.tensor.matmul(st["l4"][0:1, cq * P:(cq + 1) * P],
                                 linv4[:, cq:cq + 1], identf[:],
                                 is_transpose=True, start=True, stop=True,
                                 skip_group_check=True)
            s4 = lvp.tile([1, 512], BF16, tag="s4", bufs=2)
            nc.vector.tensor_copy(s4[:], st["l4"][0:1, :])
            st["s4"] = s4
            tailb_queue.append((h, I))

        def drain_tailb():
            while tailb_queue:
                h, I = tailb_queue.pop(0)
                st = step_state[(h, I)]
                lb_ps = ps.tile([P, 512], F32, tag="acc", bufs=3)
                nc.tensor.matmul(lb_ps[:], onesf[:], st["s4"][:],
                                 start=True, stop=True)
                lb_sb = lvp.tile([P, 512], BF16, tag="lb", bufs=2)
                nc.vector.tensor_copy(lb_sb[:], lb_ps[:])
                o_bf = obp.tile([P, 512], BF16, tag="obf")
                nc.vector.tensor_mul(o_bf[:], st["ot"][:], lb_sb[:])
                o8hi, o8lo = st["o8"]
                hi = o8hi[:, h // 2, h % 2, :]
                nc.vector.tensor_copy(hi, o_bf[:])
                nc.vector.tensor_sub(o8lo[:, h // 2, h % 2, :], o_bf[:], hi)

        def make_wo_chains(I, o8pair, split=False):
            o8hi, o8lo = o8pair
            chains = []
            p0s, p1s = [], []
            osb_group = [None]
            # smaller trailing DMA groups on the last chunk shorten the
            # copy->dispatch->transfer tail after the final matmul
            sizes = [4, 4, 4, 2, 1, 1] if I == KQ - 1 else [4, 4, 4, 4]
            gmap = {}
            s0 = 0
            for sz in sizes:
                for j in range(sz):
                    gmap[s0 + j] = (s0, sz)
                s0 += sz
            for ocb in range(FC):
                state = {}

                def part0(ocb=ocb, state=state):
                    # heads-0/1 terms: ready long before the last head's split
                    wo_ps = ps.tile([P, 512], F32, tag="acc", bufs=3)
                    state["ps"] = wo_ps
                    nc.tensor.matmul(wo_ps[:], wo8h_sb[:, 0, ocb],
                                     o8hi[:, 0], start=True,
                                     stop=False, perf_mode=DRS)
                    nc.tensor.matmul(wo_ps[:], wo8l_sb[:, 0, ocb],
                                     o8hi[:, 0], start=False,
                                     stop=False, perf_mode=DRS)
                    nc.tensor.matmul(wo_ps[:], wo8h_sb[:, 0, ocb],
                                     o8lo[:, 0], start=False,
                                     stop=False, perf_mode=DRS)

                def part1(pos, ocb=ocb, state=state):
                    wo_ps = state["ps"]
                    nc.tensor.matmul(wo_ps[:], wo8h_sb[:, 1, ocb],
                                     o8hi[:, 1], start=False,
                                     stop=False, perf_mode=DRS)
                    nc.tensor.matmul(wo_ps[:], wo8l_sb[:, 1, ocb],
                                     o8hi[:, 1], start=False,
                                     stop=False, perf_mode=DRS)
                    nc.tensor.matmul(wo_ps[:], wo8h_sb[:, 1, ocb],
                                     o8lo[:, 1], start=False,
                                     stop=True, perf_mode=DRS)
                    gs, gsz = gmap[ocb]
                    if ocb == gs:
                        osb_t = osp.tile([P, gsz, 512], BF16,
                                         tag="osb%d" % gsz)
                        osb_group[0] = osb_t
                    osb = osb_group[0]
                    # alternate copy engine so neither Act nor DVE queues up;
                    # chains draining during the last chunk's steps (o8 of
                    # chunk KQ-2) avoid Act entirely - it is exp-saturated
                    # there
                    if pos % 2 == 1 and I != KQ - 2:
                        nc.scalar.activation(osb[:, ocb - gs, :], wo_ps[:],
                                             ACTF.Copy, scale=sc32[:])
                    else:
                        nc.vector.tensor_scalar_mul(osb[:, ocb - gs, :],
                                                    wo_ps[:], 1.0 / 1024.0)
                    if ocb == gs + gsz - 1:
                        # one batched DMA per output-column-block group
                        nc.sync.dma_start(
                            out_d[:, gs:gs + gsz, I * 512:(I + 1) * 512],
                            osb[:])

                def chain(pos, part0=part0, part1=part1):
                    part0()
                    part1(pos)
                chains.append(chain)
                p0s.append(part0)
                p1s.append(part1)
            if split:
                return p0s, p1s
            return chains

        def flush_pend():
            e = pend.pop(0)
            emit_lpv(e)

        wo_drained = [0]

        def drain_wo(n):
            for i in range(min(n, len(wo_queue))):
                wo_queue.pop(0)(wo_drained[0])
                wo_drained[0] += 1

        chunk_pair = [0, 0]  # pair counter / drained count within this chunk

        def emit_step(h, I):
            npair = 2 * (I + 1) if causal else 2 * KQ
            if h == 0:
                chunk_pair[0] = 0
                chunk_pair[1] = 0
            ptot = 4 * npair - 1  # drainable pairs this chunk
            o8 = step_state.get(("o8", I))
            if o8 is None:
                o8hi_t = o8p.tile([P, 2, 2, 512], FP8, tag="o8hi")
                o8lo_t = o8p.tile([P, 2, 2, 512], FP8, tag="o8lo")
                o8 = (o8hi_t, o8lo_t)
                step_state[("o8", I)] = o8
            step_state[(h, I)] = {"o8": o8}
            for pp in range(npair):
                p8, w0e = emit_scores_exp(h, I, pp, npair)
                if pp == 1:
                    # previous step's deferred tail, then this chunk's V
                    # chains (h==0 only) - placed after this step's first two
                    # scores so PE work hides latency
                    drain_tailb()
                    while pending_v:
                        pending_v.pop(0)()
                # pace Wo chains evenly across the chunk's pairs to fill
                # exp-latency windows (safe only after the chunk-boundary
                # drain_tailb at h==0 pp==1 - chains read the previous
                # chunk's o8, whose last head is split there)
                if not (h == 0 and pp == 0):
                    chunk_pair[0] += 1
                    target = (chunk_pair[0] * 16) // ptot
                    while chunk_pair[1] < target and wo_queue:
                        drain_wo(1)
                        chunk_pair[1] += 1
                pend.append(dict(h=h, I=I, pp=pp, p8=p8, np=npair, w0e=w0e))
                if len(pend) > 1:
                    flush_pend()

        if causal:
            for c in range(KQ):
                emit_phase1_chunk(c)
                for h in range(NH):
                    emit_step(h, c)
                wo_queue.extend(make_wo_chains(c, step_state[("o8", c)]))
            while pend:
                flush_pend()
            drain_tailb()
            drain_wo(len(wo_queue))
        else:
            for c in range(KQ):
                emit_phase1_chunk(c)
            for I in range(KQ):
                for h in range(NH):
                    emit_step(h, I)
                wo_queue.extend(make_wo_chains(I, step_state[("o8", I)]))
            while pend:
                flush_pend()
            drain_tailb()
            drain_wo(len(wo_queue))

    nc.compile()
    return nc


_PROGRAMS = {}


def _get_program(S, mode):
    key = (S, mode)
    if key not in _PROGRAMS:
        _PROGRAMS[key] = build_program(S, causal=(mode == "causal"))
    return _PROGRAMS[key]


def _detect_mode(masks):
    """masks: [B, S, S]. Returns 'zeros' | 'causal' | 'general'."""
    modes = set()
    for mb in masks:
        if not np.any(mb):
            modes.add("zeros")
            continue
        S = mb.shape[0]
        iu = np.triu_indices(S, 1)
        above = mb[iu]
        low_ok = not np.any(np.tril(mb))
        if low_ok and above.size and np.all(above <= -1e8) and \
                np.all(above == above[0]):
            modes.add("causal")
        else:
            modes.add("general")
    if modes == {"zeros"}:
        return "zeros"
    if modes == {"causal"}:
        return "causal"
    return "general"


def kernel(hidden_states, attention_mask, position_ids, Wq, Wk, Wv, Wo):
    hidden_states = np.asarray(hidden_states, dtype=np.float32)
    attention_mask = np.asarray(attention_mask, dtype=np.float32)
    position_ids = np.asarray(position_ids)
    Wq = np.asarray(Wq, dtype=np.float32)
    Wk = np.asarray(Wk, dtype=np.float32)
    Wv = np.asarray(Wv, dtype=np.float32)
    Wo = np.asarray(Wo, dtype=np.float32)

    b, S, d = hidden_states.shape
    assert b == B and d == D
    masks = attention_mask.reshape(b, S, S)
    mode = _detect_mode(masks)
    assert mode in ("causal", "zeros"), f"unsupported mask mode {mode}"
    nc = _get_program(S, mode)

    BFT = ml_dtypes.bfloat16
    F8 = ml_dtypes.float8_e4m3

    # per-batch prep
    cos_b, sin_b = [], []
    inv_freq = (1.0 / (ROPE_THETA **
                       (np.arange(0, HD, 2, dtype=np.float32) / HD))
                ).astype(np.float32)
    NKBALL = S // P
    x8_b, xvh_b, xvl_b = [], [], []

    def foldx(x):
        # x: [P, FC, S] fp8 -> [P, FC//2, NKB, P, 2] stationary fold pairs,
        # tokens reversed within each 128-block (DRS stationary encoding)
        a = np.asarray(x).reshape(P, FC // 2, 2, NKBALL, P)[..., ::-1]
        return np.ascontiguousarray(a.transpose(0, 1, 3, 4, 2))

    for bi in range(b):
        xtf = np.ascontiguousarray(
            hidden_states[bi].T.reshape(FC, P, S).transpose(1, 0, 2))
        xh = xtf.astype(F8)
        xl = (xtf - xh.astype(np.float32)).astype(F8)
        x8full = np.stack([xh, xl], axis=2)  # [P, FC, 2, S]
        x8_b.append(np.ascontiguousarray(
            x8full.reshape(P, FC, 2, S // 512, 512).transpose(3, 0, 1, 2, 4)))
        xvh_b.append(foldx(xh))
        xvl_b.append(foldx(xl))
        freqs = position_ids[bi].astype(np.float32)[:, None] * inv_freq[None, :]
        emb = np.concatenate([freqs, freqs], axis=-1)  # [S, HD]
        cos_b.append(np.ascontiguousarray(np.cos(emb).T).astype(BFT))
        sin_b.append(np.ascontiguousarray(np.sin(emb).T).astype(BFT))

    identb = np.eye(P, dtype=BFT)
    identf = np.eye(P, dtype=np.float32)
    onesb = np.ones((P, 1), dtype=F8)
    onesf = np.ones((1, P), dtype=BFT)
    if mode == "causal":
        # S^T-orientation diagonal templates: mask where k > q
        kk = np.arange(P)[:, None]
        qq = np.arange(512)[None, :]
        tmpl = np.stack([
            np.where(kbl * P + kk > qq, NEG, 0.0) for kbl in range(4)
        ], axis=1).astype(F8)  # [P, 4, 512]

    def split_wf(W, gs):
        # [P, FC, NH, HD] fp8 hi + lo of 32*W, fc-pair fold interleaves
        # (even-fc, odd-fc) with columns reversed per 128-block
        wf = np.ascontiguousarray(
            (W[:, gs] * 32.0).reshape(FC, P, NH, HD).transpose(1, 0, 2, 3))
        wh = wf.astype(F8)
        wl = (wf - wh.astype(np.float32)).astype(F8)

        def fold(w):
            out = np.empty((P, FC // 2, NH, 2 * HD), dtype=F8)
            out[..., 0::2] = w[:, 0::2][..., ::-1]
            out[..., 1::2] = w[:, 1::2][..., ::-1]
            return out
        return fold(wh), fold(wl)

    in_maps = []
    for c in range(NCORES):
        bi, g = c // 4, c % 4
        gs = slice(g * DG, (g + 1) * DG)
        wqfh, wqfl = split_wf(Wq, gs)
        wkfh, wkfl = split_wf(Wk, gs)
        # wv: x32 scale, hd-REVERSED per head (so the fp8-split V tiles are
        # stored in the column-reversed order the DRS stationary read expects)
        wvr = (32.0 * Wv[:, gs]).reshape(D, NH, HD)[:, :, ::-1].reshape(D, DG)
        wvt = np.ascontiguousarray(
            wvr.reshape(FC, P, DG).transpose(1, 0, 2))
        wv8h = wvt.astype(F8)
        wv8l = (wvt - wv8h.astype(np.float32)).astype(F8)
        # wo: [hd, hpair, ocb, j, parity] fp8 hi/lo of 32*Wo, output columns
        # reversed within each 128-block (DRS stationary encoding)
        woc = (32.0 * Wo[gs, :]).reshape(2, 2, HD, FC, P)[..., ::-1]
        wo8h = woc.astype(F8)
        wo8l = (woc - wo8h.astype(np.float32)).astype(F8)
        wo8h = np.ascontiguousarray(wo8h.transpose(2, 0, 3, 4, 1))
        wo8l = np.ascontiguousarray(wo8l.transpose(2, 0, 3, 4, 1))
        m = dict(x8c=x8_b[bi], x8vh=xvh_b[bi], x8vl=xvl_b[bi],
                 wqfh=wqfh, wqfl=wqfl, wkfh=wkfh, wkfl=wkfl,
                 wv8h=wv8h, wv8l=wv8l, wo8h=wo8h, wo8l=wo8l,
                 cos=cos_b[bi], sin=sin_b[bi], identb=identb,
                 onesb8=onesb, onesf4=onesf, identf=identf)
        if mode == "causal":
            m["tmpl"] = tmpl
        in_maps.append(m)

    import os
    trace = bool(int(os.environ.get("KERNEL_TRACE", "0")))
    res = run_bass_kernel_spmd(nc, in_maps, list(range(NCORES)), trace=trace)
    global LAST_RESULTS
    LAST_RESULTS = res

    out = np.zeros((b, S, D), dtype=np.float32)
    for c in range(NCORES):
        # out dram layout [P, FC, S]: output row ocb*128 + p = [p, ocb]
        o = res.results[c]["out"].astype(np.float32)
        o = o.transpose(1, 0, 2).reshape(D, S)
        out[c // 4] += o.T
    return out


LAST_RESULTS = None



# revision 134
# speedup vs baseline: 1.0004x; 1.0004x over previous
"""Trainium2 Bass kernel for nn_BiBoAttention (B=2, S=2048, D=2048, H=16).

Sharding: 8 cores = 2 batches x 4 head-groups (4 heads of 128 dims each).

v3 design (vs v2): every GEMM runs as fp8 DoubleRowSwInterleave matmuls
(0.5 cyc/row, 256-contraction per instruction) in DUAL (hi+lo e4m3)
precision, except scores (bf16) and probs (single fp8 - softmax diffusion
damps probs noise ~2.4x, measured):
- Q/K/V projections: 8+8+8 fc-pair-fold DRS terms (w_hi*x_hi + w_lo*x_hi +
  w_hi*x_lo), host-prepped fold interleaves; V stationary is x-side
  (x8vh/x8vl fold tensors) so out lands [token, (h,hd)].
- V is split on-device into (v_hi, v_lo) fp8 kb-pair interleaves (Act copy +
  DVE subtract of the f32 PSUM) feeding PV-DRS.
- exp() writes probs DIRECTLY as fp8 e4m3 with bias -ln(16) (overflow-safe);
  l = sum_k p via ones-matmuls on the fp8 probs.
- PV: 2 DRS matmuls per kb-pair (v_hi*p + v_lo*p); diagonal pairs narrowed
  to live columns [w0e:512] (scores, template, exp, l, PV all skip the
  fully-masked belt).
- Wo: o8 split to (hi, lo) fp8 head-pair interleaves; 6 DRS matmuls per
  128-outcol block (wo_hi*o_hi + wo_lo*o_hi + wo_hi*o_lo); output written
  TRANSPOSED [D, S] (host re-transposes) so the out-DMA stays contiguous;
  4 outcol blocks batched per DMA.
- Scale plumbing: weights x32 (fp8 range), probs /16 (exp bias), o8 = 32*out,
  final osb copy descales by 1/1024.
- Wo chains paced evenly across the next chunk's score pairs (fills Act-exp
  latency windows); chunk-major x8 dram layout (1KB runs, no small-elem DMA
  penalty).
- Causal mask still accumulated into scores PSUM by the PE via template
  matmuls; l-broadcast via PE transpose + outer-product as in v2.
- Variable out-DMA groups (4,4,4,2,1,1 on the last chunk) shorten the final
  copy->dispatch->transfer tail; PSUM-reading splits/copies balanced across
  Act and DVE (Act is exp-saturated during late-chunk steps).
- Startup stream interleaves wq-lo/wk-lo quarters with the x8/wq-hi
  quarters (B-terms stop stalling); chain copies that drain during the
  exp-saturated last chunk go DVE-only; chain matmuls grouped hp0-first so
  the final drain overlaps the last head's o8 split.
- rel-err ~1.3e-2 (limit 2e-2); cost-model timeline 258.9us vs v2's 320us.
"""
import math
import ml_dtypes
import numpy as np
from contextlib import ExitStack

LOG16 = math.log(16.0)

import concourse.bass as bass
import concourse.bass_isa as bass_isa
import concourse.mybir as mybir
import concourse.tile as tile
from concourse import bacc
from concourse.bass_utils import run_bass_kernel_spmd

F32 = mybir.dt.float32
BF16 = mybir.dt.bfloat16
FP8 = mybir.dt.float8e4
DRS = mybir.MatmulPerfMode.DoubleRowSwInterleave
ALU = mybir.AluOpType
ACTF = mybir.ActivationFunctionType

B = 2
D = 2048
H = 16
HD = 128
P = 128
FC = D // P          # 16 feature chunks
NH = 4               # heads per core
DG = NH * HD         # 512 group width
NCORES = 8
ROPE_THETA = 10000.0
NEG = -200.0         # additive mask value (exp -> 0; fits fp8 e4m3)


def build_program(S, causal):
    KQ = S // 512            # q-groups of 512
    NKB = S // P             # 128-token k-blocks
    nc = bacc.Bacc("TRN2", target_bir_lowering=False, debug=False,
                   num_devices=NCORES)

    NKBALL = S // P
    x8_d = nc.declare_dram_parameter("x8c", [S // 512, P, FC, 2, 512], FP8,
                                     isOutput=False)
    x8vh_d = nc.declare_dram_parameter("x8vh", [P, FC // 2, NKBALL, P, 2], FP8,
                                       isOutput=False)
    x8vl_d = nc.declare_dram_parameter("x8vl", [P, FC // 2, NKBALL, P, 2], FP8,
                                       isOutput=False)
    wqfh_d = nc.declare_dram_parameter("wqfh", [P, FC // 2, NH, 2 * HD], FP8,
                                       isOutput=False)
    wqfl_d = nc.declare_dram_parameter("wqfl", [P, FC // 2, NH, 2 * HD], FP8,
                                       isOutput=False)
    wkfh_d = nc.declare_dram_parameter("wkfh", [P, FC // 2, NH, 2 * HD], FP8,
                                       isOutput=False)
    wkfl_d = nc.declare_dram_parameter("wkfl", [P, FC // 2, NH, 2 * HD], FP8,
                                       isOutput=False)
    wv8h_d = nc.declare_dram_parameter("wv8h", [P, FC, DG], FP8, isOutput=False)
    wv8l_d = nc.declare_dram_parameter("wv8l", [P, FC, DG], FP8, isOutput=False)
    wo8h_d = nc.declare_dram_parameter("wo8h", [P, 2, FC, P, 2], FP8,
                                       isOutput=False)
    wo8l_d = nc.declare_dram_parameter("wo8l", [P, 2, FC, P, 2], FP8,
                                       isOutput=False)
    cs_d = nc.declare_dram_parameter("cs", [P, 2, S], BF16, isOutput=False)
    id_d = nc.declare_dram_parameter("identb", [P, P], BF16, isOutput=False)
    ones_d = nc.declare_dram_parameter("onesb8", [P, 1], FP8, isOutput=False)
    idf_d = nc.declare_dram_parameter("identf", [P, P], F32, isOutput=False)
    onesf_d = nc.declare_dram_parameter("onesf4", [1, P], BF16, isOutput=False)
    if causal:
        tm_d = nc.declare_dram_parameter("tmpl", [P, 4, 512], FP8,
                                         isOutput=False)
    out_d = nc.declare_dram_parameter("out", [P, FC, S], BF16, isOutput=True)

    with tile.TileContext(nc) as tc, ExitStack() as octx:
        sb = octx.enter_context(tc.tile_pool(name="sb", bufs=1))
        xp = octx.enter_context(tc.tile_pool(name="xp", bufs=2))
        rp = octx.enter_context(tc.tile_pool(name="rp", bufs=2))
        p8p = octx.enter_context(tc.tile_pool(name="p8p", bufs=5))
        lvp = octx.enter_context(tc.tile_pool(name="lvp", bufs=2))
        o8p = octx.enter_context(tc.tile_pool(name="o8p", bufs=2))
        obp = octx.enter_context(tc.tile_pool(name="obp", bufs=3))
        qtp = octx.enter_context(tc.tile_pool(name="qtp", bufs=2))
        osp = octx.enter_context(tc.tile_pool(name="osp", bufs=4))
        ps = octx.enter_context(tc.tile_pool(name="ps", bufs=1, space="PSUM"))

        # ---- persistent SBUF ----
        # first matmul chain needs x8 chunk 0 + wqf; split those DMAs so the
        # fcp<4 part of the first chain can start early
        x80 = xp.tile([P, FC, 2, 512], FP8, tag="x8", bufs=1)
        wqfh_sb = sb.tile([P, FC // 2, NH, 2 * HD], FP8, tag="wqfh")
        wqfl_sb = sb.tile([P, FC // 2, NH, 2 * HD], FP8, tag="wqfl")
        for q4 in range(0, 4):
            fs = slice(q4 * FC // 4, (q4 + 1) * FC // 4)
            fps = slice(q4 * FC // 8, (q4 + 1) * FC // 8)
            nc.sync.dma_start(x80[:, fs], x8_d[0, :, fs])
            nc.sync.dma_start(wqfh_sb[:, fps], wqfh_d[:, fps])
            nc.sync.dma_start(wqfl_sb[:, fps], wqfl_d[:, fps])
        cs0 = rp.tile([P, 2, 512], BF16, tag="cs")
        nc.sync.dma_start(cs0[:], cs_d[:, :, 0:512])
        biasv = sb.tile([P, 1], F32, tag="biasv")
        nc.gpsimd.memset(biasv[:], -LOG16)
        sc32 = sb.tile([P, 1], F32, tag="sc32")
        nc.gpsimd.memset(sc32[:], 1.0 / 1024.0)
        wkfh_sb = sb.tile([P, FC // 2, NH, 2 * HD], FP8, tag="wkfh")
        wkfl_sb = sb.tile([P, FC // 2, NH, 2 * HD], FP8, tag="wkfl")
        for q4 in range(0, 4):
            fps = slice(q4 * FC // 8, (q4 + 1) * FC // 8)
            nc.sync.dma_start(wkfh_sb[:, fps], wkfh_d[:, fps])
            nc.sync.dma_start(wkfl_sb[:, fps], wkfl_d[:, fps])
        wv8h_sb = sb.tile([P, FC, DG], FP8, tag="wv8h")
        wv8l_sb = sb.tile([P, FC, DG], FP8, tag="wv8l")
        identb = sb.tile([P, P], BF16, tag="identb")
        nc.sync.dma_start(identb[:], id_d[:])
        onesb = sb.tile([P, 1], FP8, tag="onesb")
        nc.sync.dma_start(onesb[:], ones_d[:])
        identf = sb.tile([P, P], F32, tag="identf")
        nc.sync.dma_start(identf[:], idf_d[:])
        onesf = sb.tile([1, P], BF16, tag="onesf")
        nc.sync.dma_start(onesf[:], onesf_d[:])
        if causal:
            tmpl = sb.tile([P, 4, 512], FP8, tag="tmpl")
            nc.sync.dma_start(tmpl[:], tm_d[:])
        kt = sb.tile([P, NH, S], BF16, tag="kt")
        vhi8 = sb.tile([P, NH, NKB // 2, HD, 2], FP8, tag="vhi8")
        vlo8 = sb.tile([P, NH, NKB // 2, HD, 2], FP8, tag="vlo8")
        wo8h_sb = sb.tile([P, 2, FC, P, 2], FP8, tag="wo8h")
        wo8l_sb = sb.tile([P, 2, FC, P, 2], FP8, tag="wo8l")

        wo_started = [False]
        chunk_tiles = {0: (x80, cs0)}
        xv_tiles = {}
        qt_tiles = {}

        def prefetch_xt(c):
            if c < KQ and c not in chunk_tiles:
                x8_n = xp.tile([P, FC, 2, 512], FP8, tag="x8", bufs=1)
                nc.sync.dma_start(x8_n[:], x8_d[c])
                cs_n = rp.tile([P, 2, 512], BF16, tag="cs")
                nc.sync.dma_start(cs_n[:], cs_d[:, :, c * 512:(c + 1) * 512])
                chunk_tiles[c] = (x8_n, cs_n)

        def prefetch_xv(c):
            if c < KQ and c not in xv_tiles:
                if c == 0:
                    nc.sync.dma_start(wv8h_sb[:], wv8h_d[:])
                    nc.sync.dma_start(wv8l_sb[:], wv8l_d[:])
                xvh_n = xp.tile([P, FC // 2, 4, P, 2], FP8, tag="xvh", bufs=1)
                nc.sync.dma_start(xvh_n[:], x8vh_d[:, :, c * 4:(c + 1) * 4])
                xvl_n = xp.tile([P, FC // 2, 4, P, 2], FP8, tag="xvl", bufs=1)
                nc.sync.dma_start(xvl_n[:], x8vl_d[:, :, c * 4:(c + 1) * 4])
                xv_tiles[c] = (xvh_n, xvl_n)

        def emit_phase1_chunk(c):
            t0 = c * 512
            prefetch_xt(c)
            prefetch_xv(c)
            x8_sb, cs_sb = chunk_tiles.pop(c)
            csl = cs_sb[:, 0, :]
            ssl = cs_sb[:, 1, :]
            xvh_sb, xvl_sb = xv_tiles.pop(c)
            prefetch_xt(c + 1)
            prefetch_xv(c + 1)
            qt_c = qtp.tile([P, NH, 512], BF16, tag="qt",
                            bufs=(2 if causal else KQ))
            qt_tiles[c] = qt_c
            sq = 1.0 / (32.0 * math.sqrt(HD))
            sk = 1.0 / 32.0
            NFP = FC // 2
            terms = ((True, 0, True, False), (False, 0, False, False),
                     (True, 1, False, True))  # (use_hi, x-slot, start, stop)

            def proj_mm(qk_ap, wh_sb, wl_sb, h, fcp, term):
                use_hi, slot, st_, sp_ = terms[term]
                w_sb_ = wh_sb if use_hi else wl_sb
                nc.tensor.matmul(qk_ap, w_sb_[:, fcp, h, :],
                                 x8_sb[:, 2 * fcp:2 * fcp + 2, slot, :],
                                 start=(st_ and fcp == 0),
                                 stop=(sp_ and fcp == NFP - 1),
                                 perf_mode=DRS)

            def rope(qk_ap, h, side, ss):
                # RoPE: ro = s*qk*cos + rot_half(s*qk)*sin
                roc = rp.tile([P, 512], BF16, tag="roc")
                rot = rp.tile([P, 512], BF16, tag="rot")
                nc.vector.scalar_tensor_tensor(
                    roc[:], qk_ap, ss, csl, op0=ALU.mult, op1=ALU.mult)
                nc.vector.scalar_tensor_tensor(
                    rot[0:64, :], qk_ap[64:128, :], -ss,
                    ssl[0:64, :], op0=ALU.mult, op1=ALU.mult)
                nc.vector.scalar_tensor_tensor(
                    rot[64:128, :], qk_ap[0:64, :], ss,
                    ssl[64:128, :], op0=ALU.mult, op1=ALU.mult)
                if side == 0:
                    nc.gpsimd.tensor_add(qt_c[:, h, :], roc[:], rot[:])
                else:
                    nc.gpsimd.tensor_add(kt[:, h, t0:t0 + 512],
                                         roc[:], rot[:])

            for side, (wh_sb, wl_sb, ss) in enumerate(
                    ((wqfh_sb, wqfl_sb, sq), (wkfh_sb, wkfl_sb, sk))):
                for h in range(NH):
                    qk = ps.tile([P, 512], F32, tag="acc", bufs=3)
                    for term in range(3):
                        for fcp in range(NFP):
                            proj_mm(qk[:], wh_sb, wl_sb, h, fcp, term)
                    rope(qk[:], h, side, ss)
            def emit_v(kb2, c=c, xvh_sb=xvh_sb, xvl_sb=xvl_sb):
                kb = c * 4 + kb2
                pv = ps.tile([P, NH, HD], F32, tag="acc", bufs=3)
                for fcp in range(NFP):
                    nc.tensor.matmul(pv[:], xvh_sb[:, fcp, kb2],
                                     wv8h_sb[:, 2 * fcp:2 * fcp + 2, :],
                                     start=(fcp == 0), stop=False,
                                     perf_mode=DRS)
                for fcp in range(NFP):
                    nc.tensor.matmul(pv[:], xvh_sb[:, fcp, kb2],
                                     wv8l_sb[:, 2 * fcp:2 * fcp + 2, :],
                                     start=False, stop=False, perf_mode=DRS)
                for fcp in range(NFP):
                    nc.tensor.matmul(pv[:], xvl_sb[:, fcp, kb2],
                                     wv8h_sb[:, 2 * fcp:2 * fcp + 2, :],
                                     start=False, stop=(fcp == NFP - 1),
                                     perf_mode=DRS)
                hi = vhi8[:, :, kb // 2, :, kb % 2]
                nc.vector.tensor_copy(hi, pv[:])
                nc.vector.tensor_sub(vlo8[:, :, kb // 2, :, kb % 2], pv[:], hi)
            for kb2 in range(4):
                pending_v.append(lambda kb2=kb2: emit_v(kb2))
            if not wo_started[0]:
                wo_started[0] = True
                nc.sync.dma_start(wo8h_sb[:], wo8h_d[:])
                nc.sync.dma_start(wo8l_sb[:], wo8l_d[:])

        # ---- phase 2 ----
        pend = []          # pipelined (state dict) entries, depth 1
        wo_queue = []      # deferred Wo chain closures
        pending_v = []     # deferred V-projection chains of the current chunk
        step_state = {}

        def emit_scores_exp(h, I, pp, npair):
            s2 = ps.tile([P, 2, 512], F32, tag="s2", bufs=2)
            qsl = qt_tiles[I][:, h, :]
            diag = causal and (2 * pp >= 4 * I)
            # w0e: columns [0, w0e) of BOTH slots are fully masked for a diag
            # pair - never computed, never exp'd, and PV/l skip them.
            w0e = (2 * pp - 4 * I) * P if diag else 0
            for t in range(2):
                kb = 2 * pp + t
                if not diag:
                    nc.tensor.matmul(s2[:, t, :], kt[:, h, kb * P:(kb + 1) * P],
                                     qsl, start=True, stop=True)
                else:
                    # diag block: scores on [w0:512] only; the template matmul
                    # supplies -200 on [w0e:w0] (slot 1's leading masked belt)
                    # plus the triangle band [w0:w].
                    kbl = kb - 4 * I
                    w0 = kbl * P
                    w = w0 + P
                    nc.tensor.matmul(s2[:, t, w0:512],
                                     kt[:, h, kb * P:(kb + 1) * P],
                                     qsl[:, w0:512], start=True, stop=False)
                    nc.tensor.matmul(s2[:, t, w0e:w], identb[:],
                                     tmpl[:, kbl, w0e:w], start=False,
                                     stop=True, skip_group_check=True)
            p8 = p8p.tile([P, 2, 512], FP8, tag="p8")
            if w0e > 0:
                nc.scalar.activation(p8[:, :, w0e:512], s2[:, :, w0e:512],
                                     ACTF.Exp, bias=biasv[:])
            else:
                nc.scalar.activation(p8[:], s2[:], ACTF.Exp, bias=biasv[:])
            return p8, w0e

        def emit_lpv(e):
            h, I, pp, p8, npair = e["h"], e["I"], e["pp"], e["p8"], e["np"]
            w0e = e["w0e"]
            st = step_state[(h, I)]
            if pp == 0:
                l4_t = ps.tile([P, 512], F32, tag="l4", bufs=1)
                ot_t = ps.tile([P, 512], F32, tag="acc", bufs=3)
                st["l4"] = l4_t
                st["ot"] = ot_t
            first = (pp == 0)
            last = (pp == npair - 1)
            # l via ~1-cycle PE ones-matmuls. Only the very first matmul of the
            # step carries start=True: the executor's pending-zero marking is
            # bank-granular, so that single start arms the whole l4 bank and
            # each column's first write lands as a fresh value.
            # Quarters entirely inside [0, w0e) hold garbage p8 - skip (their
            # true contribution is zero).
            for t in range(2):
                for cq in range(4):
                    if (cq + 1) * P <= w0e:
                        continue
                    nc.tensor.matmul(
                        st["l4"][:, cq:cq + 1],
                        p8[:, t, cq * P:(cq + 1) * P], onesb[:],
                        start=(first and t == 0 and cq == 0),
                        stop=(last and t == 1 and cq == 3),
                        skip_group_check=True)
            nc.tensor.matmul(st["ot"][:, w0e:512], vhi8[:, h, pp, :, :],
                             p8[:, :, w0e:512],
                             start=first, stop=False, perf_mode=DRS,
                             skip_group_check=True)
            nc.tensor.matmul(st["ot"][:, w0e:512], vlo8[:, h, pp, :, :],
                             p8[:, :, w0e:512],
                             start=False, stop=last, perf_mode=DRS,
                             skip_group_check=True)
            if last:
                emit_norm_tail_a(h, I)

        tailb_queue = []

        def emit_norm_tail_a(h, I):
            # 1/l, then transpose it into the spare columns of the l4 bank at
            # partition quadrants (no DMA round-trip), copy to SBUF once
            st = step_state[(h, I)]
            linv4 = lvp.tile([P, 4], F32, tag="linv4")
            nc.vector.reciprocal(linv4[:], st["l4"][:, 0:4])
            for cq in range(4):
                nc.tensor.matmul(st["l4"][0:1, cq * P:(cq + 1) * P],
                                 linv4[:, cq:cq + 1], identf[:],
                                 is_transpose=True, start=True, stop=True,
                                 skip_group_check=True)
            s4 = lvp.tile([1, 512], BF16, tag="s4", bufs=2)
            nc.vector.tensor_copy(s4[:], st["l4"][0:1, :])
            st["s4"] = s4
            tailb_queue.append((h, I))

        def drain_tailb():
            while tailb_queue:
                h, I = tailb_queue.pop(0)
                st = step_state[(h, I)]
                lb_ps = ps.tile([P, 512], F32, tag="acc", bufs=3)
                nc.tensor.matmul(lb_ps[:], onesf[:], st["s4"][:],
                                 start=True, stop=True)
                lb_sb = lvp.tile([P, 512], BF16, tag="lb", bufs=2)
                nc.vector.tensor_copy(lb_sb[:], lb_ps[:])
                o_bf = obp.tile([P, 512], BF16, tag="obf")
                nc.vector.tensor_mul(o_bf[:], st["ot"][:], lb_sb[:])
                o8hi, o8lo = st["o8"]
                hi = o8hi[:, h // 2, h % 2, :]
                nc.vector.tensor_copy(hi, o_bf[:])
                nc.vector.tensor_sub(o8lo[:, h // 2, h % 2, :], o_bf[:], hi)

        def make_wo_chains(I, o8pair, split=False):
            o8hi, o8lo = o8pair
            chains = []
            p0s, p1s = [], []
            osb_group = [None]
            # smaller trailing DMA groups on the last chunk shorten the
            # copy->dispatch->transfer tail after the final matmul
            sizes = [4, 4, 4, 2, 1, 1] if I == KQ - 1 else [4, 4, 4, 4]
            gmap = {}
            s0 = 0
            for sz in sizes:
                for j in range(sz):
                    gmap[s0 + j] = (s0, sz)
                s0 += sz
            for ocb in range(FC):
                state = {}

                def part0(ocb=ocb, state=state):
                    # heads-0/1 terms: ready long before the last head's split
                    wo_ps = ps.tile([P, 512], F32, tag="acc", bufs=3)
                    state["ps"] = wo_ps
                    nc.tensor.matmul(wo_ps[:], wo8h_sb[:, 0, ocb],
                                     o8hi[:, 0], start=True,
                                     stop=False, perf_mode=DRS)
                    nc.tensor.matmul(wo_ps[:], wo8l_sb[:, 0, ocb],
                                     o8hi[:, 0], start=False,
                                     stop=False, perf_mode=DRS)
                    nc.tensor.matmul(wo_ps[:], wo8h_sb[:, 0, ocb],
                                     o8lo[:, 0], start=False,
                                     stop=False, perf_mode=DRS)

                def part1(pos, ocb=ocb, state=state):
                    wo_ps = state["ps"]
                    nc.tensor.matmul(wo_ps[:], wo8h_sb[:, 1, ocb],
                                     o8hi[:, 1], start=False,
                                     stop=False, perf_mode=DRS)
                    nc.tensor.matmul(wo_ps[:], wo8l_sb[:, 1, ocb],
                                     o8hi[:, 1], start=False,
                                     stop=False, perf_mode=DRS)
                    nc.tensor.matmul(wo_ps[:], wo8h_sb[:, 1, ocb],
                                     o8lo[:, 1], start=False,
                                     stop=True, perf_mode=DRS)
                    gs, gsz = gmap[ocb]
                    if ocb == gs:
                        osb_t = osp.tile([P, gsz, 512], BF16,
                                         tag="osb%d" % gsz)
                        osb_group[0] = osb_t
                    osb = osb_group[0]
                    # alternate copy engine so neither Act nor DVE queues up;
                    # chains draining during the last chunk's steps (o8 of
                    # chunk KQ-2) avoid Act entirely - it is exp-saturated
                    # there
                    if pos % 2 == 1 and I != KQ - 2:
                        nc.scalar.activation(osb[:, ocb - gs, :], wo_ps[:],
                                             ACTF.Copy, scale=sc32[:])
                    else:
                        nc.vector.tensor_scalar_mul(osb[:, ocb - gs, :],
                                                    wo_ps[:], 1.0 / 1024.0)
                    if ocb == gs + gsz - 1:
                        # one batched DMA per output-column-block group
                        nc.sync.dma_start(
                            out_d[:, gs:gs + gsz, I * 512:(I + 1) * 512],
                            osb[:])

                def chain(pos, part0=part0, part1=part1):
                    part0()
                    part1(pos)
                chains.append(chain)
                p0s.append(part0)
                p1s.append(part1)
            if split:
                return p0s, p1s
            return chains

        def flush_pend():
            e = pend.pop(0)
            emit_lpv(e)

        wo_drained = [0]

        def drain_wo(n):
            for i in range(min(n, len(wo_queue))):
                wo_queue.pop(0)(wo_drained[0])
                wo_drained[0] += 1

        chunk_pair = [0, 0]  # pair counter / drained count within this chunk

        def emit_step(h, I):
            npair = 2 * (I + 1) if causal else 2 * KQ
            if h == 0:
                chunk_pair[0] = 0
                chunk_pair[1] = 0
            ptot = 4 * npair - 1  # drainable pairs this chunk
            o8 = step_state.get(("o8", I))
            if o8 is None:
                o8hi_t = o8p.tile([P, 2, 2, 512], FP8, tag="o8hi")
                o8lo_t = o8p.tile([P, 2, 2, 512], FP8, tag="o8lo")
                o8 = (o8hi_t, o8lo_t)
                step_state[("o8", I)] = o8
            step_state[(h, I)] = {"o8": o8}
            for pp in range(npair):
                p8, w0e = emit_scores_exp(h, I, pp, npair)
                if pp == 1:
                    # previous step's deferred tail, then this chunk's V
                    # chains (h==0 only) - placed after this step's first two
                    # scores so PE work hides latency
                    drain_tailb()
                    while pending_v:
                        pending_v.pop(0)()
                # pace Wo chains evenly across the chunk's pairs to fill
                # exp-latency windows (safe only after the chunk-boundary
                # drain_tailb at h==0 pp==1 - chains read the previous
                # chunk's o8, whose last head is split there)
                if not (h == 0 and pp == 0):
                    chunk_pair[0] += 1
                    target = (chunk_pair[0] * 16) // ptot
                    while chunk_pair[1] < target and wo_queue:
                        drain_wo(1)
                        chunk_pair[1] += 1
                pend.append(dict(h=h, I=I, pp=pp, p8=p8, np=npair, w0e=w0e))
                if len(pend) > 1:
                    flush_pend()

        if causal:
            for c in range(KQ):
                emit_phase1_chunk(c)
                for h in range(NH):
                    emit_step(h, c)
                wo_queue.extend(make_wo_chains(c, step_state[("o8", c)]))
            while pend:
                flush_pend()
            drain_tailb()
            drain_wo(len(wo_queue))
        else:
            for c in range(KQ):
                emit_phase1_chunk(c)
            for I in range(KQ):
                for h in range(NH):
                    emit_step(h, I)
                wo_queue.extend(make_wo_chains(I, step_state[("o8", I)]))
            while pend:
                flush_pend()
            drain_tailb()
            drain_wo(len(wo_queue))

    nc.compile()
    return nc


_PROGRAMS = {}


def _get_program(S, mode):
    key = (S, mode)
    if key not in _PROGRAMS:
        _PROGRAMS[key] = build_program(S, causal=(mode == "causal"))
    return _PROGRAMS[key]


def _detect_mode(masks):
    """masks: [B, S, S]. Returns 'zeros' | 'causal' | 'general'."""
    modes = set()
    for mb in masks:
        if not np.any(mb):
            modes.add("zeros")
            continue
        S = mb.shape[0]
        iu = np.triu_indices(S, 1)
        above = mb[iu]
        low_ok = not np.any(np.tril(mb))
        if low_ok and above.size and np.all(above <= -1e8) and \
                np.all(above == above[0]):
            modes.add("causal")
        else:
            modes.add("general")
    if modes == {"zeros"}:
        return "zeros"
    if modes == {"causal"}:
        return "causal"
    return "general"


def kernel(hidden_states, attention_mask, position_ids, Wq, Wk, Wv, Wo):
    hidden_states = np.asarray(hidden_states, dtype=np.float32)
    attention_mask = np.asarray(attention_mask, dtype=np.float32)
    position_ids = np.asarray(position_ids)
    Wq = np.asarray(Wq, dtype=np.float32)
    Wk = np.asarray(Wk, dtype=np.float32)
    Wv = np.asarray(Wv, dtype=np.float32)
    Wo = np.asarray(Wo, dtype=np.float32)

    b, S, d = hidden_states.shape
    assert b == B and d == D
    masks = attention_mask.reshape(b, S, S)
    mode = _detect_mode(masks)
    assert mode in ("causal", "zeros"), f"unsupported mask mode {mode}"
    nc = _get_program(S, mode)

    BFT = ml_dtypes.bfloat16
    F8 = ml_dtypes.float8_e4m3

    # per-batch prep
    cos_b = []
    inv_freq = (1.0 / (ROPE_THETA **
                       (np.arange(0, HD, 2, dtype=np.float32) / HD))
                ).astype(np.float32)
    NKBALL = S // P
    x8_b, xvh_b, xvl_b = [], [], []

    def foldx(x):
        # x: [P, FC, S] fp8 -> [P, FC//2, NKB, P, 2] stationary fold pairs,
        # tokens reversed within each 128-block (DRS stationary encoding)
        a = np.asarray(x).reshape(P, FC // 2, 2, NKBALL, P)[..., ::-1]
        return np.ascontiguousarray(a.transpose(0, 1, 3, 4, 2))

    for bi in range(b):
        xtf = np.ascontiguousarray(
            hidden_states[bi].T.reshape(FC, P, S).transpose(1, 0, 2))
        xh = xtf.astype(F8)
        xl = (xtf - xh.astype(np.float32)).astype(F8)
        x8full = np.stack([xh, xl], axis=2)  # [P, FC, 2, S]
        x8_b.append(np.ascontiguousarray(
            x8full.reshape(P, FC, 2, S // 512, 512).transpose(3, 0, 1, 2, 4)))
        xvh_b.append(foldx(xh))
        xvl_b.append(foldx(xl))
        freqs = position_ids[bi].astype(np.float32)[:, None] * inv_freq[None, :]
        emb = np.concatenate([freqs, freqs], axis=-1)  # [S, HD]
        cos_b.append(np.ascontiguousarray(
            np.stack([np.cos(emb).T, np.sin(emb).T], axis=1)).astype(BFT))

    identb = np.eye(P, dtype=BFT)
    identf = np.eye(P, dtype=np.float32)
    onesb = np.ones((P, 1), dtype=F8)
    onesf = np.ones((1, P), dtype=BFT)
    if mode == "causal":
        # S^T-orientation diagonal templates: mask where k > q
        kk = np.arange(P)[:, None]
        qq = np.arange(512)[None, :]
        tmpl = np.stack([
            np.where(kbl * P + kk > qq, NEG, 0.0) for kbl in range(4)
        ], axis=1).astype(F8)  # [P, 4, 512]

    def split_wf(W, gs):
        # [P, FC, NH, HD] fp8 hi + lo of 32*W, fc-pair fold interleaves
        # (even-fc, odd-fc) with columns reversed per 128-block
        wf = np.ascontiguousarray(
            (W[:, gs] * 32.0).reshape(FC, P, NH, HD).transpose(1, 0, 2, 3))
        wh = wf.astype(F8)
        wl = (wf - wh.astype(np.float32)).astype(F8)

        def fold(w):
            out = np.empty((P, FC // 2, NH, 2 * HD), dtype=F8)
            out[..., 0::2] = w[:, 0::2][..., ::-1]
            out[..., 1::2] = w[:, 1::2][..., ::-1]
            return out
        return fold(wh), fold(wl)

    in_maps = []
    for c in range(NCORES):
        bi, g = c // 4, c % 4
        gs = slice(g * DG, (g + 1) * DG)
        wqfh, wqfl = split_wf(Wq, gs)
        wkfh, wkfl = split_wf(Wk, gs)
        # wv: x32 scale, hd-REVERSED per head (so the fp8-split V tiles are
        # stored in the column-reversed order the DRS stationary read expects)
        wvr = (32.0 * Wv[:, gs]).reshape(D, NH, HD)[:, :, ::-1].reshape(D, DG)
        wvt = np.ascontiguousarray(
            wvr.reshape(FC, P, DG).transpose(1, 0, 2))
        wv8h = wvt.astype(F8)
        wv8l = (wvt - wv8h.astype(np.float32)).astype(F8)
        # wo: [hd, hpair, ocb, j, parity] fp8 hi/lo of 32*Wo, output columns
        # reversed within each 128-block (DRS stationary encoding)
        woc = (32.0 * Wo[gs, :]).reshape(2, 2, HD, FC, P)[..., ::-1]
        wo8h = woc.astype(F8)
        wo8l = (woc - wo8h.astype(np.float32)).astype(F8)
        wo8h = np.ascontiguousarray(wo8h.transpose(2, 0, 3, 4, 1))
        wo8l = np.ascontiguousarray(wo8l.transpose(2, 0, 3, 4, 1))
        m = dict(x8c=x8_b[bi], x8vh=xvh_b[bi], x8vl=xvl_b[bi],
                 wqfh=wqfh, wqfl=wqfl, wkfh=wkfh, wkfl=wkfl,
                 wv8h=wv8h, wv8l=wv8l, wo8h=wo8h, wo8l=wo8l,
                 cs=cos_b[bi], identb=identb,
                 onesb8=onesb, onesf4=onesf, identf=identf)
        if mode == "causal":
            m["tmpl"] = tmpl
        in_maps.append(m)

    import os
    trace = bool(int(os.environ.get("KERNEL_TRACE", "0")))
    res = run_bass_kernel_spmd(nc, in_maps, list(range(NCORES)), trace=trace)
    global LAST_RESULTS
    LAST_RESULTS = res

    out = np.zeros((b, S, D), dtype=np.float32)
    for c in range(NCORES):
        # out dram layout [P, FC, S]: output row ocb*128 + p = [p, ocb]
        o = res.results[c]["out"].astype(np.float32)
        o = o.transpose(1, 0, 2).reshape(D, S)
        out[c // 4] += o.T
    return out


LAST_RESULTS = None

